# revision 17
# baseline (speedup 1.0000x reference)
"""Mistral attention (B=2, S=2048, D=4096, H=32, KVH=8, HD=128) on 8 trn2 cores.

Sharding: core c -> (batch b = c//4, head-group g = c%4).
Each core computes q/k/v projections for its 8 Q heads + 2 KV heads of one
batch, RoPE, causal attention, and a row-parallel partial o_proj. The
partial o_proj output is produced TRANSPOSED ([D, S]); the host transposes
and sums the 4 partials per batch. No collectives.

All matmul operands are bf16 (PSUM accumulation stays fp32).

Attention runs transposed: scoresT[keys, qtok] with keys on partitions, so
softmax uses an unstable exp (logits are O(10); fp32 exp-safe). The causal
mask is applied as an in-place [128,128] triangle add into the scores PSUM
only on the 4 diagonal sub-blocks of each query block; off-window columns
are simply never computed/read. The key-sum uses a ones[128,128] stationary
so the denominator lands replicated across partitions (no gpsimd
partition_broadcast). Attention for query block t is fused after the
projections of token block t, one head at a time (keeps PSUM to 8 banks:
2 each for proj/o_proj, scoresT, attn accumulate, key-sum).

PE idle during softmax is filled by interleaving, between attention head
groups: the K projection of block t+1 and the o_proj chunks of query block
t-1 (wo streamed per block in 2MB super-chunks). Only query block 3's
o_proj remains as a tail phase.

All input DMAs share the in-order sync HWDGE queue, so emission order ==
arrival order; the startup sequence is emitted in first-use order.
"""

import os
import sys

for _p in ("/opt/trn_rl_repo",):
    if _p not in sys.path:
        sys.path.insert(0, _p)

import numpy as np
from ml_dtypes import bfloat16

import concourse.bass as bass
import concourse.tile as tile
from concourse import bacc, mybir
from concourse.bass_utils import run_bass_kernel_spmd

F32 = mybir.dt.float32
BF16 = mybir.dt.bfloat16
EXP = mybir.ActivationFunctionType.Exp

B, S, D = 2, 2048, 4096
H, KVH, HD = 32, 8, 128
SCALE = HD ** -0.5
NCORES = 8

QH = H // 4              # 8 q heads per core
QCOLS = QH * HD          # 1024
KCOLS = (KVH // 4) * HD  # 256 (2 kv heads per core)
TOK = S
NCH = D // 128           # 32 contraction chunks
NTH = 4
THW = TOK // NTH         # 512

NEG = -1e9

_PROGRAMS = {}


def _build_program(variant: str):
    """variant: 'causal' | 'zero' | 'general'"""
    nc = bacc.Bacc("TRN2", target_bir_lowering=False, debug=False)

    hT = nc.dram_tensor("hT", [4, 2, 128, 16 * 512], BF16, kind="ExternalInput").ap()
    wq = nc.dram_tensor("wq", [8, 128, NCH * 128], BF16, kind="ExternalInput").ap()
    wk = nc.dram_tensor("wk", [128, 2 * NCH * 128], BF16, kind="ExternalInput").ap()
    wv = nc.dram_tensor("wv", [128, NCH * 256], BF16, kind="ExternalInput").ap()
    wo = nc.dram_tensor("wo", [32, 128, QH * 128], BF16, kind="ExternalInput").ap()
    cosT = nc.dram_tensor("cosT", [HD, TOK], F32, kind="ExternalInput").ap()
    sinTr = nc.dram_tensor("sinTr", [HD, TOK], F32, kind="ExternalInput").ap()
    ones = nc.dram_tensor("ones", [128, 128], BF16, kind="ExternalInput").ap()
    if variant == "causal":
        maskT = nc.dram_tensor("maskT", [128, 128], F32, kind="ExternalInput").ap()
    elif variant == "general":
        maskT = nc.dram_tensor("maskT", [S, S], F32, kind="ExternalInput").ap()
    else:
        maskT = None
    outT = nc.dram_tensor("outT", [D, TOK], BF16, kind="ExternalOutput").ap()

    if variant != "causal":
        qT_spill = nc.dram_tensor("qT_spill", [QCOLS, TOK], BF16).ap()

    with tile.TileContext(nc) as tc:
        with tc.tile_pool(name="per", bufs=1) as per, \
             tc.tile_pool(name="wrk", bufs=2) as wrk, \
             tc.tile_pool(name="one", bufs=1) as one, \
             tc.tile_pool(name="ps", bufs=2, space="PSUM") as psp:

            # ---- resident tiles ----
            wk_sb = per.tile([128, 2 * NCH * 128], BF16, tag="wk")
            wv_sb = per.tile([128, NCH * 256], BF16, tag="wv")
            ones_sb = per.tile([128, 128], BF16, tag="ones")
            kT_sb = per.tile([HD, 2 * TOK], BF16, tag="kT")
            V_sb = per.tile([128, (TOK // 128) * KCOLS], BF16, tag="V")
            if variant == "causal":
                tri_sb = per.tile([128, 128], F32, tag="tri")

            attn_sb = [one.tile([128, TOK], BF16, tag=f"at{h}", name=f"attn_{h}")
                       for h in range(QH)]

            hts = [one.tile([128, 4096], BF16, tag=f"hT{j}", name=f"hts_{j}")
                   for j in range(4)]

            def dma_hts(th):
                for j in range(4):
                    nc.sync.dma_start(
                        hts[j][:],
                        hT[th, j // 2, :, (j % 2) * 4096:(j % 2 + 1) * 4096])

            def dma_cos_sin(th):
                ts = th * THW
                cos_t = wrk.tile([HD, THW], F32, tag="cos", name=f"cos_{th}")
                sin_t = wrk.tile([HD, THW], F32, tag="sin", name=f"sin_{th}")
                nc.sync.dma_start(cos_t[:], cosT[:, ts:ts + THW])
                nc.sync.dma_start(sin_t[:], sinTr[:, ts:ts + THW])
                return cos_t, sin_t

            # ---- startup DMA sequence, first-use order; first chunks are
            # split fine so the very first matmuls can start early ----
            nc.sync.dma_start(wk_sb[:, 0:1024], wk[:, 0:1024])
            nc.sync.dma_start(hts[0][:, 0:2048], hT[0, 0, :, 0:2048])
            nc.sync.dma_start(wk_sb[:, 1024:2048], wk[:, 1024:2048])
            nc.sync.dma_start(hts[0][:, 2048:4096], hT[0, 0, :, 2048:4096])
            nc.sync.dma_start(wk_sb[:, 2048:4096], wk[:, 2048:4096])
            nc.sync.dma_start(hts[1][:], hT[0, 0, :, 4096:8192])
            cos0, sin0 = dma_cos_sin(0)
            nc.sync.dma_start(hts[2][:], hT[0, 1, :, 0:4096])
            nc.sync.dma_start(hts[3][:], hT[0, 1, :, 4096:8192])
            nc.sync.dma_start(wk_sb[:, 4096:6144], wk[:, 4096:6144])
            nc.sync.dma_start(wk_sb[:, 6144:8192], wk[:, 6144:8192])
            for j in range(4):
                nc.sync.dma_start(
                    wv_sb[:, j * 2048:(j + 1) * 2048], wv[:, j * 2048:(j + 1) * 2048])
            nc.sync.dma_start(ones_sb[:], ones[:])
            if variant == "causal":
                nc.sync.dma_start(tri_sb[:], maskT[:])

            def rope(ps, dst, cos_t, sin_t):
                # out = x*cos + swap_halves(x)*sin_signed
                m1 = wrk.tile([128, THW], F32, tag="m1")
                nc.vector.tensor_mul(m1[:, :THW], ps[:], cos_t[:])
                m2 = wrk.tile([128, THW], F32, tag="m2")
                nc.vector.tensor_mul(m2[0:64, :], ps[64:128, :], sin_t[0:64, :])
                nc.vector.tensor_mul(m2[64:128, :], ps[0:64, :], sin_t[64:128, :])
                nc.vector.tensor_add(dst, m1[:, :THW], m2[:])

            def kproj(th, cb, cos_t, sin_t):
                ts = th * THW
                ps = psp.tile([128, THW], F32, tag="pa", name=f"kp_{th}_{cb}")
                for ic in range(NCH):
                    nc.tensor.matmul(
                        ps[:],
                        wk_sb[:, (cb * NCH + ic) * 128:(cb * NCH + ic + 1) * 128],
                        hts[ic // 8][:, (ic % 8) * THW:(ic % 8 + 1) * THW],
                        start=(ic == 0), stop=(ic == NCH - 1))
                rope(ps, kT_sb[:, cb * TOK + ts: cb * TOK + ts + THW], cos_t, sin_t)

            def vproj(th):
                for tc4 in range(4):
                    v_ps = psp.tile([128, 512], F32, tag="pa", name=f"vp_{th}_{tc4}")
                    for ic in range(NCH):
                        nc.tensor.matmul(
                            v_ps[:, :256],
                            hts[ic // 8][:, (ic % 8) * THW + tc4 * 128:
                                         (ic % 8) * THW + (tc4 + 1) * 128],
                            wv_sb[:, ic * 256:(ic + 1) * 256],
                            start=(ic == 0), stop=(ic == NCH - 1))
                    tb = th * 4 + tc4
                    nc.scalar.copy(
                        V_sb[:, tb * KCOLS:(tb + 1) * KCOLS], v_ps[:, :256])

            def qproj(th, qT_lo, qT_hi, cos_t, sin_t):
                tiles = {}
                def fire(cb):
                    w_sb = wrk.tile([128, NCH * 128], BF16, tag="w", bufs=3,
                                    name=f"wq_{th}_{cb}")
                    if th == 0 and cb == 0:
                        nc.sync.dma_start(w_sb[:, :2048], wq[cb, :, :2048])
                        nc.sync.dma_start(w_sb[:, 2048:], wq[cb, :, 2048:])
                    else:
                        nc.sync.dma_start(w_sb[:], wq[cb])
                    tiles[cb] = w_sb
                fire(0); fire(1); fire(2)
                for cb in range(QH):
                    w_sb = tiles.pop(cb)
                    ps = psp.tile([128, THW], F32, tag="pa", name=f"qp_{th}_{cb}")
                    for ic in range(NCH):
                        nc.tensor.matmul(
                            ps[:],
                            w_sb[:, ic * 128:(ic + 1) * 128],
                            hts[ic // 8][:, (ic % 8) * THW:(ic % 8 + 1) * THW],
                            start=(ic == 0), stop=(ic == NCH - 1))
                    if cb + 3 < QH:
                        fire(cb + 3)
                    qdst = qT_lo if cb < 4 else qT_hi
                    rope(ps, qdst[:, (cb % 4) * 512:(cb % 4 + 1) * 512],
                         cos_t, sin_t)

            def oproj_pair(qb, oc0, wo_sb):
                """Two o_proj output chunks (oc0, oc0+1) for query block qb,
                staged into one tile and written with a single DMA (fewer
                DMAs -> shorter end-of-program semaphore teardown). Copy and
                DMA both on the Act engine: the DIRECT2D issues in program
                order right after the copies (data ready, no head-of-line
                wait), and output writes never block input loads on the
                in-order sync queue."""
                ot = wrk.tile([128, 1024], BF16, tag="ot", bufs=2,
                              name=f"ot_{oc0}_{qb}")
                for i in range(2):
                    oc = oc0 + i
                    o_ps = psp.tile([128, 512], F32, tag="pa",
                                    name=f"o_{oc}_{qb}")
                    ol = oc % 8
                    for hc in range(QH):
                        nc.tensor.matmul(
                            o_ps[:],
                            wo_sb[:, ol * 1024 + hc * 128:
                                  ol * 1024 + (hc + 1) * 128],
                            attn_sb[hc][:, qb * 512:(qb + 1) * 512],
                            start=(hc == 0), stop=(hc == QH - 1))
                    nc.scalar.copy(ot[:, i * 512:(i + 1) * 512], o_ps[:])
                nc.scalar.dma_start(
                    outT[oc0 * 128:(oc0 + 2) * 128, qb * 512:(qb + 1) * 512]
                    .rearrange("(o p) c -> p o c", p=128),
                    ot[:].rearrange("p (o c) -> p o c", o=2))

            def dma_wo_chunk(sc):
                wo_sb = wrk.tile([128, 8 * 1024], BF16, tag="wo", name=f"wo_{sc}")
                nc.sync.dma_start(
                    wo_sb[:].rearrange("p (o c) -> p o c", o=8),
                    wo[8 * (sc % 4):8 * (sc % 4) + 8].rearrange("o p c -> p o c"))
                return wo_sb

            def attention_head(h, qb, qT_ap, filler):
                """Causal attention for one q head / query block, transposed
                orientation. `filler` is a list of thunks; one is popped and
                emitted right before the final AV (covering the last exp
                latency)."""
                qs = qb * 512
                nkb = 4 * qb + 4 if variant == "causal" else TOK // 128
                kv = h // (QH // 2)
                att_ps = psp.tile([128, 512], F32, tag="pc", name=f"att_{h}_{qb}")
                sum_ps = psp.tile([128, 512], F32, tag="pd", name=f"sum_{h}_{qb}")

                def co_of(kb):
                    if variant == "causal" and kb > 4 * qb:
                        return (kb - 4 * qb) * 128
                    return 0

                def emit_av(kb, expT):
                    co = co_of(kb)
                    nc.tensor.matmul(
                        att_ps[:, co:],
                        V_sb[:, kb * KCOLS + kv * 128: kb * KCOLS + (kv + 1) * 128],
                        expT[:, co:],
                        start=(kb == 0), stop=(kb == nkb - 1))
                    nc.tensor.matmul(
                        sum_ps[:, co:], ones_sb[:], expT[:, co:],
                        start=(kb == 0), stop=(kb == nkb - 1))

                pend = None
                for kb in range(nkb):
                    co = co_of(kb)
                    s_w = psp.tile([128, 512], F32, tag="pb",
                                   name=f"s_{h}_{qb}_{kb}")
                    nc.tensor.matmul(
                        s_w[:, co:],
                        kT_sb[:, kv * TOK + kb * 128: kv * TOK + (kb + 1) * 128],
                        qT_ap[:, co:],
                        start=True, stop=True)
                    if variant == "causal" and kb >= 4 * qb:
                        # triangle window: keys of this block vs cols [co,co+128)
                        nc.vector.tensor_add(
                            s_w[:, co:co + 128], s_w[:, co:co + 128], tri_sb[:])
                    elif variant == "general":
                        mt = wrk.tile([128, 512], F32, tag="mt",
                                      name=f"mt_{h}_{qb}_{kb}")
                        nc.sync.dma_start(
                            mt[:], maskT[kb * 128:(kb + 1) * 128, qs:qs + 512])
                        nc.vector.tensor_add(s_w[:], s_w[:], mt[:])
                    expT = wrk.tile([128, 512], BF16, tag="expT", bufs=4,
                                    name=f"exp_{h}_{qb}_{kb}")
                    nc.scalar.activation(
                        expT[:, co:], s_w[:, co:], EXP, scale=float(SCALE))
                    if pend is not None:
                        emit_av(*pend)
                    pend = (kb, expT)
                if filler:
                    filler.pop(0)()
                emit_av(*pend)
                rb = wrk.tile([128, 512], F32, tag="rcp", name=f"rcp_{h}_{qb}")
                nc.vector.reciprocal_approx_fast(rb[:], sum_ps[:])
                nc.vector.tensor_mul(
                    attn_sb[h][:, qs:qs + 512], att_ps[:], rb[:])

            # ================= main loop =================
            qT_lo = one.tile([128, 4 * 512], BF16, tag="qTbl")
            qT_hi = one.tile([128, 4 * 512], BF16, tag="qTbh")

            # warmup: dependency-free matmuls on SBUF garbage run during the
            # initial DMA prefix and ramp the PE out of its low p-state, so
            # the first real matmuls issue at full clock.
            for wu in range(20):
                wps = psp.tile([128, 512], F32, tag="pb", name=f"wu_{wu}")
                nc.tensor.matmul(
                    wps[:], attn_sb[0][:, 0:128], attn_sb[1][:, 0:512],
                    start=True, stop=True)

            cs = {0: (cos0, sin0)}
            for th in range(NTH):
                cos_t, sin_t = cs.pop(th)
                if th == 0:
                    kproj(0, 0, cos_t, sin_t)
                    kproj(0, 1, cos_t, sin_t)
                vproj(th)
                qproj(th, qT_lo, qT_hi, cos_t, sin_t)

                if variant == "causal":
                    # prefetch for next block + wo stream for this block's
                    # interleaved o_proj
                    if th + 1 < NTH:
                        dma_hts(th + 1)
                        cs[th + 1] = dma_cos_sin(th + 1)
                    wo_tiles = {}
                    if th >= 1:
                        wo_tiles[0] = dma_wo_chunk(0)
                        wo_tiles[1] = dma_wo_chunk(1)

                    # filler units to interleave between attention heads
                    filler = []
                    if th + 1 < NTH:
                        ncs, nsn = cs[th + 1]
                        filler.append(lambda c=ncs, s=nsn, t=th:
                                      kproj(t + 1, 0, c, s))
                        filler.append(lambda c=ncs, s=nsn, t=th:
                                      kproj(t + 1, 1, c, s))
                    if th >= 1:
                        def mk(oc0):
                            def f():
                                sc = oc0 // 8
                                if sc + 1 < 4 and (sc + 1) not in wo_tiles and \
                                        oc0 % 8 == 4:
                                    wo_tiles[sc + 1] = dma_wo_chunk(sc + 1)
                                oproj_pair(th - 1, oc0, wo_tiles[sc])
                            return f
                        for oc0 in range(0, 32, 2):
                            filler.append(mk(oc0))

                    # spread filler units across the 8 head groups
                    per_head = [[] for _ in range(QH)]
                    for i, f in enumerate(filler):
                        per_head[(i * QH) // len(filler)].append(f)
                    for h in range(QH):
                        qsrc = qT_lo if h < 4 else qT_hi
                        qT_ap = qsrc[:, (h % 4) * 512:(h % 4 + 1) * 512]
                        attention_head(h, th, qT_ap, per_head[h])
                        # emit any remaining filler for this head slot
                        for f in per_head[h]:
                            f()
                        per_head[h] = []
                else:
                    for qi, qt in ((0, qT_lo), (1, qT_hi)):
                        nc.sync.dma_start(
                            qT_spill[qi * 512:(qi + 1) * 512,
                                     th * THW:th * THW + THW]
                            .rearrange("(i p) t -> p i t", p=128),
                            qt[:].rearrange("p (i t) -> p i t", i=4),
                        )
                    if th + 1 < NTH:
                        dma_hts(th + 1)
                        cs[th + 1] = dma_cos_sin(th + 1)
                        kproj(th + 1, 0, *cs[th + 1])
                        kproj(th + 1, 1, *cs[th + 1])

            if variant != "causal":
                for h in range(QH):
                    for qb in range(4):
                        qT_t = wrk.tile([128, 512], BF16, tag="qTs",
                                        name=f"qt_{h}_{qb}")
                        nc.sync.dma_start(
                            qT_t[:],
                            qT_spill[h * 128:(h + 1) * 128,
                                     qb * 512:(qb + 1) * 512])
                        attention_head(h, qb, qT_t[:], [])
                qbs = range(4)
            else:
                qbs = [3]

            # ---- o_proj tail ----
            for qb in qbs:
                for sc in range(4):
                    wo_sb = dma_wo_chunk(sc)
                    for ol in range(0, 8, 2):
                        oproj_pair(qb, sc * 8 + ol, wo_sb)

    nc.compile()
    return nc


def _get_program(variant: str):
    if variant not in _PROGRAMS:
        _PROGRAMS[variant] = _build_program(variant)
    return _PROGRAMS[variant]


def _detect_variant(mask: np.ndarray) -> str:
    m = mask.reshape(mask.shape[-2], mask.shape[-1])
    if not m.any():
        return "zero"
    causal = np.where(
        np.tril(np.ones((S, S), dtype=bool)), np.float32(0.0), np.float32(NEG))
    if np.array_equal(m, causal):
        return "causal"
    return "general"


def kernel(hidden_states, cos, sin, attention_mask, Wq, Wk, Wv, Wo):
    hidden_states = np.asarray(hidden_states, dtype=np.float32)
    cos = np.asarray(cos, dtype=np.float32)
    sin = np.asarray(sin, dtype=np.float32)
    attention_mask = np.asarray(attention_mask, dtype=np.float32)
    Wq = np.asarray(Wq, dtype=np.float32)
    Wk = np.asarray(Wk, dtype=np.float32)
    Wv = np.asarray(Wv, dtype=np.float32)
    Wo = np.asarray(Wo, dtype=np.float32)

    variant = _detect_variant(attention_mask)
    nc = _get_program(variant)

    ones = np.ones((128, 128), dtype=bfloat16)

    if variant == "causal":
        i = np.arange(128)[:, None]
        j = np.arange(128)[None, :]
        maskT = np.where(i <= j, np.float32(0.0),
                         np.float32(NEG / SCALE)).astype(np.float32)
    elif variant == "general":
        m = attention_mask.reshape(S, S)
        maskT = np.ascontiguousarray(m.T / np.float32(SCALE))
    else:
        maskT = None

    per_batch = {}
    for b in range(B):
        sT = np.ascontiguousarray(sin[b].T)
        sinTr = np.concatenate([-sT[:64], sT[64:]], axis=0)
        hid = hidden_states[b]  # [2048, 4096]
        hT_t = np.ascontiguousarray(
            hid.reshape(4, 512, 2, 16, 128).transpose(0, 2, 4, 3, 1)
            .reshape(4, 2, 128, 16 * 512)).astype(bfloat16)
        per_batch[b] = (hT_t, np.ascontiguousarray(cos[b].T),
                        np.ascontiguousarray(sinTr))

    in_maps = []
    for c in range(NCORES):
        b, g = divmod(c, 4)
        hT_t, cosT_a, sinTr_a = per_batch[b]
        wq_c = Wq[:, g * QCOLS:(g + 1) * QCOLS]       # [4096, 1024]
        wq_t = np.ascontiguousarray(
            wq_c.reshape(NCH, 128, 8, 128).transpose(2, 1, 0, 3)
            .reshape(8, 128, NCH * 128)).astype(bfloat16)
        wk_c = Wk[:, g * KCOLS:(g + 1) * KCOLS]       # [4096, 256]
        wk_t = np.ascontiguousarray(
            wk_c.reshape(NCH, 128, 2, 128).transpose(1, 2, 0, 3)
            .reshape(128, 2 * NCH * 128)).astype(bfloat16)
        wv_c = Wv[:, g * KCOLS:(g + 1) * KCOLS]       # [4096, 256]
        wv_t = np.ascontiguousarray(
            wv_c.reshape(NCH, 128, 256).transpose(1, 0, 2)
            .reshape(128, NCH * 256)).astype(bfloat16)
        wo_c = Wo[g * QCOLS:(g + 1) * QCOLS, :]       # [1024, 4096]
        wo_t = np.ascontiguousarray(
            wo_c.reshape(8, 128, 32, 128).transpose(2, 1, 0, 3)
            .reshape(32, 128, 8 * 128)).astype(bfloat16)
        im = {
            "hT": hT_t,
            "wq": wq_t,
            "wk": wk_t,
            "wv": wv_t,
            "wo": wo_t,
            "cosT": cosT_a,
            "sinTr": sinTr_a,
            "ones": ones,
        }
        if maskT is not None:
            im["maskT"] = maskT
        in_maps.append(im)

    trace = bool(os.environ.get("KERNEL_TRACE"))
    res = run_bass_kernel_spmd(nc, in_maps, core_ids=list(range(NCORES)),
                               trace=trace)
    if trace:
        print(f"HW exec time: {res.exec_time_ns} ns")
        kernel.last_result = res

    out = np.empty((B, S, D), dtype=np.float32)
    for b in range(B):
        acc = np.zeros((S, D), dtype=np.float64)
        for g in range(4):
            acc += res.results[4 * b + g]["outT"].astype(np.float32).T
        out[b] = acc.astype(np.float32)
    return out


# revision 22
# speedup vs baseline: 1.1071x; 1.1071x over previous
"""Mistral attention (B=2, S=2048, D=4096, H=32, KVH=8, HD=128) on 8 trn2 cores.

Sharding: core c -> (batch b = c//4, head-group g = c%4).
Each core computes q/k/v projections for its 8 Q heads + 2 KV heads of one
batch, RoPE, causal attention, and a row-parallel partial o_proj. The
partial o_proj output is produced TRANSPOSED ([D, S]); the host transposes
and sums the 4 partials per batch. No collectives.

All matmul operands are bf16 (PSUM accumulation stays fp32).

Attention runs transposed: scoresT[keys, qtok] with keys on partitions, so
softmax uses an unstable exp (logits are O(10); fp32 exp-safe). The causal
mask is applied as an in-place [128,128] triangle add into the scores PSUM
only on the 4 diagonal sub-blocks of each query block; off-window columns
are simply never computed/read. The key-sum uses a ones[128,128] stationary
so the denominator lands replicated across partitions (no gpsimd
partition_broadcast). Attention for query block t is fused after the
projections of token block t, one head at a time (keeps PSUM to 8 banks:
2 each for proj/o_proj, scoresT, attn accumulate, key-sum).

PE idle during softmax is filled by interleaving, between attention head
groups: the K projection of block t+1 and the o_proj chunks of query block
t-1 (wo streamed per block in 2MB super-chunks). Only query block 3's
o_proj remains as a tail phase.

All input DMAs share the in-order sync HWDGE queue, so emission order ==
arrival order; the startup sequence is emitted in first-use order.
"""

import os
import sys

for _p in ("/opt/trn_rl_repo",):
    if _p not in sys.path:
        sys.path.insert(0, _p)

import numpy as np
from ml_dtypes import bfloat16

import concourse.bass as bass
import concourse.tile as tile
from concourse import bacc, mybir
from concourse.bass_utils import run_bass_kernel_spmd

F32 = mybir.dt.float32
BF16 = mybir.dt.bfloat16
EXP = mybir.ActivationFunctionType.Exp

B, S, D = 2, 2048, 4096
H, KVH, HD = 32, 8, 128
SCALE = HD ** -0.5
NCORES = 8

QH = H // 4              # 8 q heads per core
QCOLS = QH * HD          # 1024
KCOLS = (KVH // 4) * HD  # 256 (2 kv heads per core)
TOK = S
NCH = D // 128           # 32 contraction chunks
NTH = 4
THW = TOK // NTH         # 512

NEG = -1e9

_PROGRAMS = {}


def _build_program(variant: str):
    """variant: 'causal' | 'zero' | 'general'"""
    nc = bacc.Bacc("TRN2", target_bir_lowering=False, debug=False)

    hT = nc.dram_tensor("hT", [4, 2, 128, 16 * 512], BF16, kind="ExternalInput").ap()
    wq = nc.dram_tensor("wq", [8, 128, NCH * 128], BF16, kind="ExternalInput").ap()
    wk = nc.dram_tensor("wk", [128, 2 * NCH * 128], BF16, kind="ExternalInput").ap()
    wv = nc.dram_tensor("wv", [128, NCH * 256], BF16, kind="ExternalInput").ap()
    wo = nc.dram_tensor("wo", [32, 128, QH * 128], BF16, kind="ExternalInput").ap()
    cosT = nc.dram_tensor("cosT", [HD, TOK], F32, kind="ExternalInput").ap()
    sinTr = nc.dram_tensor("sinTr", [HD, TOK], F32, kind="ExternalInput").ap()
    ones = nc.dram_tensor("ones", [128, 128], BF16, kind="ExternalInput").ap()
    if variant == "causal":
        maskT = nc.dram_tensor("maskT", [128, 128], F32, kind="ExternalInput").ap()
    elif variant == "general":
        maskT = nc.dram_tensor("maskT", [S, S], F32, kind="ExternalInput").ap()
    else:
        maskT = None
    outT = nc.dram_tensor("outT", [D, TOK], BF16, kind="ExternalOutput").ap()

    if variant != "causal":
        qT_spill = nc.dram_tensor("qT_spill", [QCOLS, TOK], BF16).ap()

    with tile.TileContext(nc) as tc:
        with tc.tile_pool(name="per", bufs=1) as per, \
             tc.tile_pool(name="wrk", bufs=2) as wrk, \
             tc.tile_pool(name="one", bufs=1) as one, \
             tc.tile_pool(name="ps", bufs=2, space="PSUM") as psp:

            # ---- resident tiles ----
            wk_sb = per.tile([128, 2 * NCH * 128], BF16, tag="wk")
            wv_sb = per.tile([128, NCH * 256], BF16, tag="wv")
            ones_sb = per.tile([128, 128], BF16, tag="ones")
            kT_sb = per.tile([HD, 2 * TOK], BF16, tag="kT")
            V_sb = per.tile([128, (TOK // 128) * KCOLS], BF16, tag="V")
            if variant == "causal":
                tri_sb = per.tile([128, 128], F32, tag="tri")

            attn_sb = [one.tile([128, TOK], BF16, tag=f"at{h}", name=f"attn_{h}")
                       for h in range(QH)]

            hts = [one.tile([128, 4096], BF16, tag=f"hT{j}", name=f"hts_{j}")
                   for j in range(4)]

            def dma_hts(th):
                for j in range(4):
                    nc.sync.dma_start(
                        hts[j][:],
                        hT[th, j // 2, :, (j % 2) * 4096:(j % 2 + 1) * 4096])

            def dma_cos_sin(th):
                ts = th * THW
                cos_t = wrk.tile([HD, THW], F32, tag="cos", name=f"cos_{th}")
                sin_t = wrk.tile([HD, THW], F32, tag="sin", name=f"sin_{th}")
                nc.sync.dma_start(cos_t[:], cosT[:, ts:ts + THW])
                nc.sync.dma_start(sin_t[:], sinTr[:, ts:ts + THW])
                return cos_t, sin_t

            # ---- startup DMA sequence, first-use order; first chunks are
            # split fine so the very first matmuls can start early ----
            nc.sync.dma_start(wk_sb[:, 0:1024], wk[:, 0:1024])
            nc.sync.dma_start(hts[0][:, 0:2048], hT[0, 0, :, 0:2048])
            nc.sync.dma_start(wk_sb[:, 1024:2048], wk[:, 1024:2048])
            nc.sync.dma_start(hts[0][:, 2048:4096], hT[0, 0, :, 2048:4096])
            nc.sync.dma_start(wk_sb[:, 2048:4096], wk[:, 2048:4096])
            for j in range(1, 4):
                nc.sync.dma_start(
                    hts[j][:],
                    hT[0, j // 2, :, (j % 2) * 4096:(j % 2 + 1) * 4096])
            cos0, sin0 = dma_cos_sin(0)
            nc.sync.dma_start(wk_sb[:, 4096:6144], wk[:, 4096:6144])
            nc.sync.dma_start(wk_sb[:, 6144:8192], wk[:, 6144:8192])
            nc.sync.dma_start(ones_sb[:], ones[:])
            if variant == "causal":
                nc.sync.dma_start(tri_sb[:], maskT[:])
            for j in range(4):
                nc.sync.dma_start(
                    wv_sb[:, j * 2048:(j + 1) * 2048], wv[:, j * 2048:(j + 1) * 2048])

            def rope(ps, dst, cos_t, sin_t):
                # out = x*cos + swap_halves(x)*sin_signed
                m1 = wrk.tile([128, THW], F32, tag="m1")
                nc.vector.tensor_mul(m1[:, :THW], ps[:], cos_t[:])
                m2 = wrk.tile([128, THW], F32, tag="m2")
                nc.vector.tensor_mul(m2[0:64, :], ps[64:128, :], sin_t[0:64, :])
                nc.vector.tensor_mul(m2[64:128, :], ps[0:64, :], sin_t[64:128, :])
                nc.vector.tensor_add(dst, m1[:, :THW], m2[:])

            def kproj(th, cb, cos_t, sin_t):
                ts = th * THW
                ps = psp.tile([128, THW], F32, tag="pa", name=f"kp_{th}_{cb}")
                for ic in range(NCH):
                    nc.tensor.matmul(
                        ps[:],
                        wk_sb[:, (cb * NCH + ic) * 128:(cb * NCH + ic + 1) * 128],
                        hts[ic // 8][:, (ic % 8) * THW:(ic % 8 + 1) * THW],
                        start=(ic == 0), stop=(ic == NCH - 1))
                rope(ps, kT_sb[:, cb * TOK + ts: cb * TOK + ts + THW], cos_t, sin_t)

            def vproj(th):
                for tc4 in range(4):
                    v_ps = psp.tile([128, 512], F32, tag="pa", name=f"vp_{th}_{tc4}")
                    for ic in range(NCH):
                        nc.tensor.matmul(
                            v_ps[:, :256],
                            hts[ic // 8][:, (ic % 8) * THW + tc4 * 128:
                                         (ic % 8) * THW + (tc4 + 1) * 128],
                            wv_sb[:, ic * 256:(ic + 1) * 256],
                            start=(ic == 0), stop=(ic == NCH - 1))
                    tb = th * 4 + tc4
                    nc.scalar.copy(
                        V_sb[:, tb * KCOLS:(tb + 1) * KCOLS], v_ps[:, :256])

            def qproj(th, qT_lo, qT_hi, cos_t, sin_t):
                tiles = {}
                def fire(cb):
                    w_sb = wrk.tile([128, NCH * 128], BF16, tag="w", bufs=3,
                                    name=f"wq_{th}_{cb}")
                    nc.sync.dma_start(w_sb[:, :2048], wq[cb, :, :2048])
                    nc.sync.dma_start(w_sb[:, 2048:], wq[cb, :, 2048:])
                    tiles[cb] = w_sb
                fire(0); fire(1); fire(2)
                for cb in range(QH):
                    w_sb = tiles.pop(cb)
                    ps = psp.tile([128, THW], F32, tag="pa", name=f"qp_{th}_{cb}")
                    for ic in range(NCH):
                        nc.tensor.matmul(
                            ps[:],
                            w_sb[:, ic * 128:(ic + 1) * 128],
                            hts[ic // 8][:, (ic % 8) * THW:(ic % 8 + 1) * THW],
                            start=(ic == 0), stop=(ic == NCH - 1))
                    if cb + 3 < QH:
                        fire(cb + 3)
                    qdst = qT_lo if cb < 4 else qT_hi
                    rope(ps, qdst[:, (cb % 4) * 512:(cb % 4 + 1) * 512],
                         cos_t, sin_t)

            def oproj_chunk(qb, oc, wo_sb):
                """One o_proj output chunk (128 rows of outT) for query block
                qb; wo_sb holds 8 oc chunks, use oc%8."""
                o_ps = psp.tile([128, 512], F32, tag="pa", name=f"o_{oc}_{qb}")
                ol = oc % 8
                for hc in range(QH):
                    nc.tensor.matmul(
                        o_ps[:],
                        wo_sb[:, ol * 1024 + hc * 128: ol * 1024 + (hc + 1) * 128],
                        attn_sb[hc][:, qb * 512:(qb + 1) * 512],
                        start=(hc == 0), stop=(hc == QH - 1))
                # Copy and DMA both on the Act engine: the DIRECT2D issues in
                # program order right after the copy (data already ready, so
                # no head-of-line wait on the Act HWDGE queue), and output
                # writes never block input loads on the in-order sync queue.
                ot = wrk.tile([128, 512], BF16, tag="ot", bufs=4,
                              name=f"ot_{oc}_{qb}")
                nc.scalar.copy(ot[:], o_ps[:])
                nc.scalar.dma_start(
                    outT[oc * 128:(oc + 1) * 128, qb * 512:(qb + 1) * 512],
                    ot[:])

            def dma_wo_chunk(sc):
                wo_sb = wrk.tile([128, 8 * 1024], BF16, tag="wo", name=f"wo_{sc}")
                nc.sync.dma_start(
                    wo_sb[:].rearrange("p (o c) -> p o c", o=8),
                    wo[8 * (sc % 4):8 * (sc % 4) + 8].rearrange("o p c -> p o c"))
                return wo_sb

            def attention_head(h, qb, qT_ap, filler):
                """Causal attention for one q head / query block, transposed
                orientation. `filler` is a list of thunks; one is popped and
                emitted right before the final AV (covering the last exp
                latency)."""
                qs = qb * 512
                nkb = 4 * qb + 4 if variant == "causal" else TOK // 128
                kv = h // (QH // 2)
                att_ps = psp.tile([128, 512], F32, tag="pc", name=f"att_{h}_{qb}")
                sum_ps = psp.tile([128, 512], F32, tag="pd", name=f"sum_{h}_{qb}")

                def co_of(kb):
                    if variant == "causal" and kb > 4 * qb:
                        return (kb - 4 * qb) * 128
                    return 0

                def emit_av(kb, expT):
                    co = co_of(kb)
                    nc.tensor.matmul(
                        att_ps[:, co:],
                        V_sb[:, kb * KCOLS + kv * 128: kb * KCOLS + (kv + 1) * 128],
                        expT[:, co:],
                        start=(kb == 0), stop=(kb == nkb - 1))
                    nc.tensor.matmul(
                        sum_ps[:, co:], ones_sb[:], expT[:, co:],
                        start=(kb == 0), stop=(kb == nkb - 1))

                pend = None
                for kb in range(nkb):
                    co = co_of(kb)
                    s_w = psp.tile([128, 512], F32, tag="pb",
                                   name=f"s_{h}_{qb}_{kb}")
                    nc.tensor.matmul(
                        s_w[:, co:],
                        kT_sb[:, kv * TOK + kb * 128: kv * TOK + (kb + 1) * 128],
                        qT_ap[:, co:],
                        start=True, stop=True)
                    if variant == "causal" and kb >= 4 * qb:
                        # triangle window: keys of this block vs cols [co,co+128)
                        nc.vector.tensor_add(
                            s_w[:, co:co + 128], s_w[:, co:co + 128], tri_sb[:])
                    elif variant == "general":
                        mt = wrk.tile([128, 512], F32, tag="mt",
                                      name=f"mt_{h}_{qb}_{kb}")
                        nc.sync.dma_start(
                            mt[:], maskT[kb * 128:(kb + 1) * 128, qs:qs + 512])
                        nc.vector.tensor_add(s_w[:], s_w[:], mt[:])
                    expT = wrk.tile([128, 512], BF16, tag="expT", bufs=4,
                                    name=f"exp_{h}_{qb}_{kb}")
                    nc.scalar.activation(
                        expT[:, co:], s_w[:, co:], EXP, scale=float(SCALE))
                    if pend is not None:
                        emit_av(*pend)
                    pend = (kb, expT)
                if filler:
                    filler.pop(0)()
                emit_av(*pend)
                rb = wrk.tile([128, 512], F32, tag="rcp", name=f"rcp_{h}_{qb}")
                nc.vector.reciprocal_approx_fast(rb[:], sum_ps[:])
                nc.vector.tensor_mul(
                    attn_sb[h][:, qs:qs + 512], att_ps[:], rb[:])

            # ================= main loop =================
            qT_lo = one.tile([128, 4 * 512], BF16, tag="qTbl")
            qT_hi = one.tile([128, 4 * 512], BF16, tag="qTbh")

            # warmup: dependency-free matmuls on SBUF garbage run during the
            # initial DMA prefix and ramp the PE out of its low p-state, so
            # the first real matmuls issue at full clock.
            for wu in range(20):
                wps = psp.tile([128, 512], F32, tag="pb", name=f"wu_{wu}")
                nc.tensor.matmul(
                    wps[:], attn_sb[0][:, 0:128], attn_sb[1][:, 0:512],
                    start=True, stop=True)

            cs = {0: (cos0, sin0)}
            for th in range(NTH):
                cos_t, sin_t = cs.pop(th)
                if th == 0:
                    kproj(0, 0, cos_t, sin_t)
                    kproj(0, 1, cos_t, sin_t)
                vproj(th)
                qproj(th, qT_lo, qT_hi, cos_t, sin_t)

                if variant == "causal":
                    # prefetch for next block + wo stream for this block's
                    # interleaved o_proj
                    if th + 1 < NTH:
                        dma_hts(th + 1)
                        cs[th + 1] = dma_cos_sin(th + 1)
                    wo_tiles = {}
                    if th >= 1:
                        wo_tiles[0] = dma_wo_chunk(0)
                        wo_tiles[1] = dma_wo_chunk(1)

                    # filler units to interleave between attention heads
                    filler = []
                    if th + 1 < NTH:
                        ncs, nsn = cs[th + 1]
                        filler.append(lambda c=ncs, s=nsn, t=th:
                                      kproj(t + 1, 0, c, s))
                        filler.append(lambda c=ncs, s=nsn, t=th:
                                      kproj(t + 1, 1, c, s))
                    if th >= 1:
                        def mk(oc):
                            def f():
                                sc = oc // 8
                                if sc + 1 < 4 and (sc + 1) not in wo_tiles and \
                                        oc % 8 == 4:
                                    wo_tiles[sc + 1] = dma_wo_chunk(sc + 1)
                                oproj_chunk(th - 1, oc, wo_tiles[sc])
                            return f
                        for oc in range(32):
                            filler.append(mk(oc))

                    # spread filler units across the 8 head groups
                    per_head = [[] for _ in range(QH)]
                    for i, f in enumerate(filler):
                        per_head[(i * QH) // len(filler)].append(f)
                    for h in range(QH):
                        qsrc = qT_lo if h < 4 else qT_hi
                        qT_ap = qsrc[:, (h % 4) * 512:(h % 4 + 1) * 512]
                        attention_head(h, th, qT_ap, per_head[h])
                        # emit any remaining filler for this head slot
                        for f in per_head[h]:
                            f()
                        per_head[h] = []
                else:
                    for qi, qt in ((0, qT_lo), (1, qT_hi)):
                        nc.sync.dma_start(
                            qT_spill[qi * 512:(qi + 1) * 512,
                                     th * THW:th * THW + THW]
                            .rearrange("(i p) t -> p i t", p=128),
                            qt[:].rearrange("p (i t) -> p i t", i=4),
                        )
                    if th + 1 < NTH:
                        dma_hts(th + 1)
                        cs[th + 1] = dma_cos_sin(th + 1)
                        kproj(th + 1, 0, *cs[th + 1])
                        kproj(th + 1, 1, *cs[th + 1])

            if variant != "causal":
                for h in range(QH):
                    for qb in range(4):
                        qT_t = wrk.tile([128, 512], BF16, tag="qTs",
                                        name=f"qt_{h}_{qb}")
                        nc.sync.dma_start(
                            qT_t[:],
                            qT_spill[h * 128:(h + 1) * 128,
                                     qb * 512:(qb + 1) * 512])
                        attention_head(h, qb, qT_t[:], [])
                qbs = range(4)
            else:
                qbs = [3]

            # ---- o_proj tail ----
            for qb in qbs:
                for sc in range(4):
                    wo_sb = dma_wo_chunk(sc)
                    for ol in range(8):
                        oproj_chunk(qb, sc * 8 + ol, wo_sb)

    nc.compile()
    return nc


def _get_program(variant: str):
    if variant not in _PROGRAMS:
        _PROGRAMS[variant] = _build_program(variant)
    return _PROGRAMS[variant]


def _detect_variant(mask: np.ndarray) -> str:
    m = mask.reshape(mask.shape[-2], mask.shape[-1])
    if not m.any():
        return "zero"
    causal = np.where(
        np.tril(np.ones((S, S), dtype=bool)), np.float32(0.0), np.float32(NEG))
    if np.array_equal(m, causal):
        return "causal"
    return "general"


def kernel(hidden_states, cos, sin, attention_mask, Wq, Wk, Wv, Wo):
    hidden_states = np.asarray(hidden_states, dtype=np.float32)
    cos = np.asarray(cos, dtype=np.float32)
    sin = np.asarray(sin, dtype=np.float32)
    attention_mask = np.asarray(attention_mask, dtype=np.float32)
    Wq = np.asarray(Wq, dtype=np.float32)
    Wk = np.asarray(Wk, dtype=np.float32)
    Wv = np.asarray(Wv, dtype=np.float32)
    Wo = np.asarray(Wo, dtype=np.float32)

    variant = _detect_variant(attention_mask)
    nc = _get_program(variant)

    ones = np.ones((128, 128), dtype=bfloat16)

    if variant == "causal":
        i = np.arange(128)[:, None]
        j = np.arange(128)[None, :]
        maskT = np.where(i <= j, np.float32(0.0),
                         np.float32(NEG / SCALE)).astype(np.float32)
    elif variant == "general":
        m = attention_mask.reshape(S, S)
        maskT = np.ascontiguousarray(m.T / np.float32(SCALE))
    else:
        maskT = None

    per_batch = {}
    for b in range(B):
        sT = np.ascontiguousarray(sin[b].T)
        sinTr = np.concatenate([-sT[:64], sT[64:]], axis=0)
        hid = hidden_states[b]  # [2048, 4096]
        hT_t = np.ascontiguousarray(
            hid.reshape(4, 512, 2, 16, 128).transpose(0, 2, 4, 3, 1)
            .reshape(4, 2, 128, 16 * 512)).astype(bfloat16)
        per_batch[b] = (hT_t, np.ascontiguousarray(cos[b].T),
                        np.ascontiguousarray(sinTr))

    in_maps = []
    for c in range(NCORES):
        b, g = divmod(c, 4)
        hT_t, cosT_a, sinTr_a = per_batch[b]
        wq_c = Wq[:, g * QCOLS:(g + 1) * QCOLS]       # [4096, 1024]
        wq_t = np.ascontiguousarray(
            wq_c.reshape(NCH, 128, 8, 128).transpose(2, 1, 0, 3)
            .reshape(8, 128, NCH * 128)).astype(bfloat16)
        wk_c = Wk[:, g * KCOLS:(g + 1) * KCOLS]       # [4096, 256]
        wk_t = np.ascontiguousarray(
            wk_c.reshape(NCH, 128, 2, 128).transpose(1, 2, 0, 3)
            .reshape(128, 2 * NCH * 128)).astype(bfloat16)
        wv_c = Wv[:, g * KCOLS:(g + 1) * KCOLS]       # [4096, 256]
        wv_t = np.ascontiguousarray(
            wv_c.reshape(NCH, 128, 256).transpose(1, 0, 2)
            .reshape(128, NCH * 256)).astype(bfloat16)
        wo_c = Wo[g * QCOLS:(g + 1) * QCOLS, :]       # [1024, 4096]
        wo_t = np.ascontiguousarray(
            wo_c.reshape(8, 128, 32, 128).transpose(2, 1, 0, 3)
            .reshape(32, 128, 8 * 128)).astype(bfloat16)
        im = {
            "hT": hT_t,
            "wq": wq_t,
            "wk": wk_t,
            "wv": wv_t,
            "wo": wo_t,
            "cosT": cosT_a,
            "sinTr": sinTr_a,
            "ones": ones,
        }
        if maskT is not None:
            im["maskT"] = maskT
        in_maps.append(im)

    trace = bool(os.environ.get("KERNEL_TRACE"))
    res = run_bass_kernel_spmd(nc, in_maps, core_ids=list(range(NCORES)),
                               trace=trace)
    if trace:
        print(f"HW exec time: {res.exec_time_ns} ns")
        kernel.last_result = res

    out = np.empty((B, S, D), dtype=np.float32)
    for b in range(B):
        acc = np.zeros((S, D), dtype=np.float64)
        for g in range(4):
            acc += res.results[4 * b + g]["outT"].astype(np.float32).T
        out[b] = acc.astype(np.float32)
    return out
